# revision 35
# baseline (speedup 1.0000x reference)
"""KGAttentionLayer Trainium2 kernel (v7).

Cost-model engine busy per core: ACT 117us / DVE 115us / PE 112us
(PE ~82us real: the sim does not credit row-packed score concurrency) /
Pool 60us -- down from v5's DVE 144 / ACT 140 / PE 128. The measured
4-core AllGather is ~19us for the 1.06MB payload and overlaps compute.

v7 adds position-sharded k/v projections with a 4-core AllGather:
- Sharding: 8 cores = (batch 2) x (query-block 4). Core c handles batch
  b=c//4, query rows [j*512, (j+1)*512) of that batch (j=c%4). The host
  rolls x per core so its queries AND its k/v "home" positions are
  columns [0, 512).
- Each core computes k/v projections only for its 512 home positions
  (all 16 heads), stages them to DRAM, and a rank-ordered AllGather over
  the batch group [[0..3],[4..7]] reassembles the full 2048-position k/v
  on every core (softmax is permutation-invariant over attended
  positions, and rank order == absolute order, so no per-core control
  flow is needed). kg-derived k/v (256 positions) are computed locally
  on every core (cheap). This removes the 4x duplication of k/v work:
  PE busy drops ~40us, and the ACT/DVE bias/copy stages shrink 4x.
- Elementwise work is split across ACT/DVE/Pool. The GPSIMD/Pool engine
  cannot touch PSUM (BIR verifier), so it takes the SBUF-only work:
  final gate/residual chain, recr copies, vslab ones. The 18.9M-element
  exp stream runs on ACT (true exp) and DVE (Schraudolph: the e4m3 bit
  pattern of exp(x) is int8(x*8/ln2 + 55.04), +-3% which cancels in the
  softmax ratio), split 9/9 per 18 m-chunks.
- fp8e4m3 DoubleRow matmuls for all projections and attn@v (weights
  pre-scaled 16x on host to clear e4m3 subnormals; 1/16 folded into the
  bias stage; for v the 16 is cancelled by a 16.0 ones-column in vslab
  that also emits the softmax denominator row).
- v-bias is NOT applied in vslab: out = num/den + bv, so bv folds into
  the out-proj/gate biases on the host (Wo@bv, Wg1@bv); kg rows carry
  (bkv - bv) baked in on-device.
- Row-packed K=64 score matmuls: head pairs stacked on partitions
  0:64/64:128 run concurrently in separate PE row groups.

Layouts (host pre-transposes; the PE never transposes on device):
  xT      [1024, 512]   x[b].T home slice (rolled so the core's block is
                        at columns [0, 512))
  qts2    [128, 8, 512]  pair-stacked q (even head dims at parts 0:64)
  ktaG    [128, 8, 2304] pair-stacked k, all pairs x all positions
                         (cols 0:2048 gathered, 2048:2304 local kg)
  vslabG  [128, 18, 16*68] per-head 64 v-cols plus a 16.0 ones column
          -> the attn@v matmul emits the softmax numerator in PSUM rows
          0..63 and the denominator in row 64.
"""

import sys

sys.path.insert(0, "/opt/trn_rl_repo")

import numpy as np

import concourse.bass as bass
import concourse.mybir as mybir
import concourse.tile as tile
from concourse import bacc
from concourse.bass_utils import run_bass_kernel_spmd

F32 = mybir.dt.float32
BF16 = mybir.dt.bfloat16
FP8 = mybir.dt.float8e4
DR = mybir.MatmulPerfMode.DoubleRow
AF = mybir.ActivationFunctionType
OP = mybir.AluOpType

WSCALE = 16.0
RSCALE = 1.0 / WSCALE

D = 1024
H = 16
HD = 64
B = 2
L = 2048
E = 256
LBLK = 512          # queries (and home k/v positions) per core
M = L + E           # 2304 attended positions
NMC = M // 128      # 18 m-chunks
HMC = LBLK // 128   # 4 home m-chunks
VW = 68             # vslab per-head slot (65 used; padded for alignment)
N_CORES = 8
GROUPS = [[0, 1, 2, 3], [4, 5, 6, 7]]

# Per-mc exp engine assignment: 'A' = ACT true exp, 'D' = DVE Schraudolph.
EXP_PAT = ['D', 'A', 'D', 'A', 'D', 'A', 'D', 'A', 'D',
           'A', 'D', 'A', 'D', 'A', 'D', 'A', 'D', 'A']
assert len(EXP_PAT) == NMC

_CACHE = {}


def _build(repeat=1):
    nc = bacc.Bacc("TRN2", target_bir_lowering=False, debug=False,
                   num_devices=N_CORES)

    dram = {}

    def din(name, shape, dt=BF16):
        dram[name] = nc.dram_tensor(name, shape, dt, kind="ExternalInput")
        return dram[name]

    xT = din("xT", [D, LBLK], FP8)
    xR = din("xR", [128, 8, LBLK], BF16)   # residual slice of x
    kgT = din("kgT", [D, E], FP8)
    WqT = din("WqT", [D, D], FP8)
    WkT = din("WkT", [D, D], FP8)
    WkkT = din("WkkT", [D, D], FP8)
    WvT = din("WvT", [D, D], FP8)
    WkvT = din("WkvT", [D, D], FP8)
    WoT = din("WoT", [D, D], FP8)
    WgT = din("WgT", [D, D], FP8)
    bq = din("bq", [128, 8], F32)      # col g = (bias*0.125)[g*128:(g+1)*128]
    bk = din("bk", [128, 8], F32)
    bkk = din("bkk", [128, 8], F32)
    bo = din("bo", [128, 8], F32)      # bo + Wo @ bv (v-bias folded on host)
    bge = din("bge", [128, 8], F32)    # bg + Wg[:,D:] @ kg_mean + Wg[:,:D] @ bv
    bkvb = din("bkvb", [128, D], F32)  # np.tile((bkv - bv) * 16, (128, 1))

    OUTT = nc.dram_tensor("OUTT", [D, LBLK], F32, kind="ExternalOutput")

    # DRAM scratch for the k/v exchange: one fp8 payload carrying both
    # the home k (8 pairs x 512 positions, as 16*(k+bk)) and the home
    # vslab (4 m-chunks x 16*VW).
    KW = 8 * LBLK                  # 4096 kta bytes/partition
    SW = KW + HMC * H * VW         # + 4*1088 vslab bytes -> 8448
    kv_out = nc.dram_tensor("kv_out", [128, SW], FP8)
    kv_g = nc.dram_tensor("kv_g", [4, 128, SW], FP8)

    def w8(pool, W, g, tag="w8"):
        t = pool.tile([128, 8, 128], FP8, tag=tag, name=f"{tag}_{W.name}_{g}")
        nc.sync.dma_start(
            t[:], W.ap().rearrange("(kk p) d -> p kk d", p=128)
            [:, :, g * 128:(g + 1) * 128])
        return t

    def wv4(pool, W, di, tag="wv"):
        t = pool.tile([128, 8, 256], FP8, tag=tag, name=f"{tag}_{W.name}_{di}")
        nc.gpsimd.dma_start(
            t[:], W.ap().rearrange("(kk p) d -> p kk d", p=128)
            [:, :, di * 256:(di + 1) * 256])
        return t

    from contextlib import ExitStack

    with tile.TileContext(nc) as tc, ExitStack() as ctx:
        persist = ctx.enter_context(tc.tile_pool(name="persist", bufs=1))
        kvpool = ctx.enter_context(tc.tile_pool(name="kvpool", bufs=2))
        wpool = ctx.enter_context(tc.tile_pool(name="wpool", bufs=2))
        spool = ctx.enter_context(tc.tile_pool(name="spool", bufs=2))
        epool = ctx.enter_context(tc.tile_pool(name="epool", bufs=3))
        # PSUM: scores 2x[128,1024] (4 banks) + av 1x[65,1024] (2 banks)
        #       + proj 2x[128,512] (2 banks) = 8 banks
        psS = ctx.enter_context(tc.tile_pool(name="psS", bufs=2, space="PSUM"))
        psAV = ctx.enter_context(tc.tile_pool(name="psAV", bufs=1, space="PSUM"))
        psP = ctx.enter_context(tc.tile_pool(name="psP", bufs=2, space="PSUM"))

        # ---- resident loads ----
        xts = persist.tile([128, 8, LBLK], FP8, tag="xts")
        nc.sync.dma_start(xts[:], xT.ap().rearrange("(kk p) l -> p kk l", p=128))
        xrs = persist.tile([128, 8, LBLK], BF16, tag="xrs")
        nc.sync.dma_start(xrs[:], xR.ap())
        kgts = persist.tile([128, 8, E], FP8, tag="kgts")
        nc.sync.dma_start(kgts[:], kgT.ap().rearrange("(kk p) e -> p kk e", p=128))
        biases = {}
        for nm in ("bq", "bk", "bkk", "bo", "bge"):
            t = persist.tile([128, 8], F32, tag=nm, name=nm + "_sb")
            nc.sync.dma_start(t[:], dram[nm].ap())
            biases[nm] = t
        bkvbs = persist.tile([128, D], F32, tag="bkvbs")
        nc.sync.dma_start(bkvbs[:], bkvb.ap())
        onesvH = persist.tile([128, HMC, H, 1], FP8, tag="onesvH")
        nc.vector.memset(onesvH[:], WSCALE)
        onesvE = persist.tile([128, 2, H, 1], FP8, tag="onesvE")
        nc.vector.memset(onesvE[:], WSCALE)
        ones1 = persist.tile([1, 64], BF16, tag="ones1")
        nc.vector.memset(ones1[:], 1.0)

        qts2 = persist.tile([128, 8, LBLK], BF16, tag="qts2")
        outTs = persist.tile([128, 8, LBLK], FP8, tag="outTs")

        # repeat>1 builds a NEFF that runs the whole body `repeat` times --
        # used only by the timing harness (slope removes the RPC overhead).
        for _rep in range(repeat):
            # ---- home k projection: 8 pairs x home 512 positions ----
            # k is exchanged and kept in fp8 as 16*(k+bk); the 1/16 is
            # folded into the q scale, keeping e4m3 values normal.
            ktaH = kvpool.tile([128, 8, LBLK], FP8, tag="ktaH")
            for g in range(8):
                wk = w8(wpool, WkT, g, tag="wk")
                ps = psP.tile([128, LBLK], F32, tag="psP", name=f"khp{g}")
                for t in range(4):
                    nc.tensor.matmul(ps[:], wk[:, 2 * t:2 * t + 2, :],
                                     xts[:, 2 * t:2 * t + 2, :],
                                     start=(t == 0), stop=(t == 3),
                                     perf_mode=DR)
                nc.scalar.activation(ktaH[:, g, :], ps[:], AF.Identity,
                                     bias=biases["bk"][:, g:g + 1])

            # ---- home v projection: 16 heads x home positions ----
            vslabH = kvpool.tile([128, HMC, H * VW], FP8, tag="vslabH")
            nc.gpsimd.tensor_copy(
                vslabH[:].rearrange("p mc (h c) -> p mc h c", c=VW)
                [:, :, :, 64:65], onesvH[:])
            for di in range(4):
                wv = wv4(wpool, WvT, di)
                for mc in range(HMC):
                    ps = psP.tile([128, 256], F32, tag="psP",
                                  name=f"vhp{di}_{mc}")
                    for t in range(4):
                        nc.tensor.matmul(
                            ps[:], xts[:, 2 * t:2 * t + 2,
                                       mc * 128:(mc + 1) * 128],
                            wv[:, 2 * t:2 * t + 2, :],
                            start=(t == 0), stop=(t == 3), perf_mode=DR)
                    nc.vector.tensor_copy(
                        vslabH[:, mc, :].rearrange("p (h c) -> p h c", c=VW)
                        [:, 4 * di:4 * di + 4, 0:64],
                        ps[:].rearrange("p (h c) -> p h c", c=64))

            # ---- stage home k/v and gather across the batch group ----
            nc.sync.dma_start(
                kv_out.ap()[:, 0:KW].rearrange("p (g m) -> p g m", g=8),
                ktaH[:])
            nc.sync.dma_start(
                kv_out.ap()[:, KW:SW].rearrange("p (c w) -> p c w", c=HMC),
                vslabH[:])
            nc.gpsimd.collective_compute(
                "AllGather", OP.bypass, replica_groups=GROUPS,
                ins=[kv_out.ap()], outs=[kv_g.ap()])

            # ---- local kg projections (all cores, cheap) ----
            ktaG = kvpool.tile([128, 8, M], FP8, tag="ktaG")
            vslabG = kvpool.tile([128, NMC, H * VW], FP8, tag="vslabG")
            for g in range(8):
                wkk = w8(wpool, WkkT, g, tag="wk")
                ps = psP.tile([128, E], F32, tag="psP", name=f"kkp{g}")
                for t in range(4):
                    nc.tensor.matmul(ps[:], wkk[:, 2 * t:2 * t + 2, :],
                                     kgts[:, 2 * t:2 * t + 2, :],
                                     start=(t == 0), stop=(t == 3),
                                     perf_mode=DR)
                nc.scalar.activation(ktaG[:, g, L:M], ps[:], AF.Identity,
                                     bias=biases["bkk"][:, g:g + 1])
            nc.gpsimd.tensor_copy(
                vslabG[:, 16:18, :].rearrange("p mc (h c) -> p mc h c", c=VW)
                [:, :, :, 64:65], onesvE[:])
            for di in range(4):
                wkv = wv4(wpool, WkvT, di, tag="wkv")
                for emc in range(2):
                    ps = psP.tile([128, 256], F32, tag="psP",
                                  name=f"kvp{di}_{emc}")
                    for t in range(4):
                        nc.tensor.matmul(
                            ps[:], kgts[:, 2 * t:2 * t + 2,
                                        emc * 128:(emc + 1) * 128],
                            wkv[:, 2 * t:2 * t + 2, :],
                            start=(t == 0), stop=(t == 3), perf_mode=DR)
                    nc.vector.tensor_add(
                        vslabG[:, 16 + emc, :]
                        .rearrange("p (h c) -> p h c", c=VW)
                        [:, 4 * di:4 * di + 4, 0:64],
                        ps[:].rearrange("p (h c) -> p h c", c=64),
                        bkvbs[:, 256 * di:256 * di + 256]
                        .rearrange("p (h c) -> p h c", c=64))

            # ---- read back the gathered k/v (rank order == absolute) ----
            # gpsimd queue: keeps the readbacks off the sync queue (which
            # carries the weight streams) so attention can start as soon
            # as the gather lands.
            for r in range(4):
                nc.gpsimd.dma_start(
                    ktaG[:, :, r * LBLK:(r + 1) * LBLK],
                    kv_g.ap()[r][:, 0:KW].rearrange("p (g m) -> p g m", g=8))
                nc.gpsimd.dma_start(
                    vslabG[:, 4 * r:4 * r + 4, :],
                    kv_g.ap()[r][:, KW:SW].rearrange("p (c w) -> p c w",
                                                     c=HMC))

            # ---- q projections (all 8 pairs) ----
            for g in range(8):
                wq = w8(wpool, WqT, g, tag="wq")
                ps = psP.tile([128, LBLK], F32, tag="psP", name=f"qp{g}")
                for t in range(4):
                    nc.tensor.matmul(ps[:], wq[:, 2 * t:2 * t + 2, :],
                                     xts[:, 2 * t:2 * t + 2, :],
                                     start=(t == 0), stop=(t == 3),
                                     perf_mode=DR)
                # scale = attn 0.125 / 16 (Wq prescale) / 16 (kta carries
                # 16*(k+bk)); host scales bq to match
                nc.scalar.activation(qts2[:, g, :], ps[:], AF.Identity,
                                     bias=biases["bq"][:, g:g + 1],
                                     scale=0.125 * RSCALE * RSCALE)

            # ---- attention per pair g (heads 2g, 2g+1) ----
            for g in range(8):
                avp = psAV.tile([65, 1024], F32, tag="psAV", name=f"av{g}")
                for mc in range(NMC):
                    sp = psS.tile([128, 1024], F32, tag="psS",
                                  name=f"sp{g}_{mc}")
                    # two concurrent K=64 matmuls in different PE row groups
                    nc.tensor.matmul(sp[:, 0:512],
                                     ktaG[0:64, g, mc * 128:(mc + 1) * 128],
                                     qts2[0:64, g, :], start=True, stop=True)
                    nc.tensor.matmul(sp[:, 512:1024],
                                     ktaG[64:128, g, mc * 128:(mc + 1) * 128],
                                     qts2[64:128, g, :], start=True, stop=True)
                    if mc % 2 == 0:
                        et2 = epool.tile([128, 2, 1024], FP8, tag="et",
                                         name=f"et{g}_{mc}")
                    if EXP_PAT[mc] == 'A':
                        nc.scalar.activation(et2[:, mc % 2, :], sp[:], AF.Exp)
                    else:
                        nc.vector.tensor_scalar(
                            et2[:, mc % 2, :].bitcast(mybir.dt.uint8),
                            sp[:], 11.5416, 55.04, OP.mult, OP.add)
                    if mc % 2 == 1:
                        # DoubleRow fp8 av over the two chunks of et2
                        nc.tensor.matmul(
                            avp[:, 0:512],
                            vslabG[:, mc - 1:mc + 1,
                                   2 * g * VW:2 * g * VW + 65],
                            et2[:, :, 0:512],
                            start=(mc == 1), stop=(mc == NMC - 1),
                            perf_mode=DR)
                        nc.tensor.matmul(
                            avp[:, 512:1024],
                            vslabG[:, mc - 1:mc + 1,
                                   (2 * g + 1) * VW:(2 * g + 1) * VW + 65],
                            et2[:, :, 512:1024],
                            start=(mc == 1), stop=(mc == NMC - 1),
                            perf_mode=DR)
                # post-process: reciprocal of denominators, broadcast via PE,
                # normalize on DVE into outTs.
                # (reciprocal_approx_fast reads garbage from PSUM on HW --
                # stage the denominator row in SBUF first.)
                den = spool.tile([1, 1024], F32, tag="den", name=f"den{g}")
                nc.scalar.activation(den[:], avp[64:65, :], AF.Identity)
                rec = spool.tile([1, 1024], F32, tag="rec", name=f"rec{g}")
                nc.vector.reciprocal_approx_fast(rec[:], den[:])
                recr = spool.tile([1, 1024], BF16, tag="recr", name=f"recr{g}")
                nc.gpsimd.tensor_copy(recr[:], rec[:])
                avs = spool.tile([64, 1024], F32, tag="avs", name=f"avs{g}")
                nc.scalar.activation(avs[:], avp[0:64, :], AF.Identity)
                rpE = psP.tile([64, 512], F32, tag="psP", name=f"rpE{g}")
                nc.tensor.matmul(rpE[:], ones1[:], recr[:, 0:512],
                                 start=True, stop=True)
                rpO = psP.tile([64, 512], F32, tag="psP", name=f"rpO{g}")
                nc.tensor.matmul(rpO[:], ones1[:], recr[:, 512:1024],
                                 start=True, stop=True)
                nc.vector.tensor_mul(outTs[0:64, g, :], avs[:, 0:512], rpE[:])
                nc.vector.tensor_mul(outTs[64:128, g, :], avs[:, 512:1024],
                                     rpO[:])

            # ---- out-proj + gate + residual ----
            for g in range(8):
                wo = w8(wpool, WoT, g, tag="wo")
                wg = w8(wpool, WgT, g, tag="wo")
                pp = psP.tile([128, LBLK], F32, tag="psP", name=f"pp{g}")
                for t in range(4):
                    nc.tensor.matmul(pp[:], wo[:, 2 * t:2 * t + 2, :],
                                     outTs[:, 2 * t:2 * t + 2, :],
                                     start=(t == 0), stop=(t == 3),
                                     perf_mode=DR)
                pj = spool.tile([128, LBLK], F32, tag="pj", name=f"pj{g}")
                nc.scalar.activation(pj[:], pp[:], AF.Identity,
                                     bias=biases["bo"][:, g:g + 1],
                                     scale=RSCALE)
                gp = psP.tile([128, LBLK], F32, tag="psP", name=f"gp{g}")
                for t in range(4):
                    nc.tensor.matmul(gp[:], wg[:, 2 * t:2 * t + 2, :],
                                     outTs[:, 2 * t:2 * t + 2, :],
                                     start=(t == 0), stop=(t == 3),
                                     perf_mode=DR)
                gt = spool.tile([128, LBLK], F32, tag="gt", name=f"gt{g}")
                nc.scalar.activation(gt[:], gp[:], AF.Sigmoid,
                                     bias=biases["bge"][:, g:g + 1],
                                     scale=RSCALE)
                # all-SBUF chain -> Pool engine (cannot touch PSUM)
                d1 = spool.tile([128, LBLK], F32, tag="fin", name=f"d1{g}")
                nc.gpsimd.tensor_sub(d1[:], pj[:], xrs[:, g, :])
                d2 = spool.tile([128, LBLK], F32, tag="fin", name=f"d2{g}")
                nc.gpsimd.tensor_mul(d2[:], d1[:], gt[:])
                fo = spool.tile([128, LBLK], F32, tag="fin", name=f"fo{g}")
                nc.gpsimd.tensor_add(fo[:], d2[:], xrs[:, g, :])
                nc.sync.dma_start(OUTT.ap()[g * 128:(g + 1) * 128, :], fo[:])

    nc.compile()
    return nc


def kernel(x, kg_embeds, Wq, bq, Wk, bk, Wv, bv, Wkk, bkk, Wkv, bkv,
           Wo, bo, Wg, bg):
    import ml_dtypes
    bf16 = ml_dtypes.bfloat16
    f8 = ml_dtypes.float8_e4m3

    x = np.asarray(x, np.float32)
    kg_embeds = np.asarray(kg_embeds, np.float32)
    ws = {k: np.asarray(v, np.float32) for k, v in dict(
        Wq=Wq, bq=bq, Wk=Wk, bk=bk, Wv=Wv, bv=bv, Wkk=Wkk, bkk=bkk,
        Wkv=Wkv, bkv=bkv, Wo=Wo, bo=bo, Wg=Wg, bg=bg).items()}

    if "nc" not in _CACHE:
        _CACHE["nc"] = _build()
    nc = _CACHE["nc"]

    def col8(v):
        return np.ascontiguousarray(v.reshape(8, 128).T)

    def w16(w):
        # weights pre-scaled by WSCALE=16 before fp8 so they stay normal
        return np.ascontiguousarray((w.T * 16.0).astype(f8))

    shared = {
        "WqT": w16(ws["Wq"]),
        "WkT": w16(ws["Wk"]),
        "WkkT": w16(ws["Wkk"]),
        "WvT": w16(ws["Wv"]),
        "WkvT": w16(ws["Wkv"]),
        "WoT": w16(ws["Wo"]),
        "WgT": w16(ws["Wg"][:, :D]),
        "bq": col8(ws["bq"] * (0.125 / WSCALE)),
        "bk": col8(ws["bk"] * WSCALE),
        "bkk": col8(ws["bkk"] * WSCALE),
        # vslab holds 16*(x@Wv.T) WITHOUT the v bias; bv is applied
        # exactly via the out-proj/gate biases:
        #   out = num/den + bv  =>  Wo@out = Wo@(num/den) + Wo@bv
        # kg rows carry the residual (bkv - bv) baked into vslab.
        "bo": col8(ws["bo"] + ws["Wo"] @ ws["bv"]),
        "bkvb": np.ascontiguousarray(
            np.tile((ws["bkv"] - ws["bv"]) * 16.0, (128, 1))),
    }

    in_maps = []
    for c in range(N_CORES):
        b, j = divmod(c, 4)
        # roll the core's query block (== its home k/v positions) to
        # columns [0, 512); only those columns of x are needed on-device
        xb = np.ascontiguousarray(np.roll(x[b].T, -j * LBLK, axis=1)[:, :LBLK])
        kgm = kg_embeds[b].mean(axis=0)
        bge = ws["bg"] + ws["Wg"][:, D:] @ kgm + ws["Wg"][:, :D] @ ws["bv"]
        m = dict(shared)
        m["xT"] = xb.astype(f8)
        m["xR"] = np.ascontiguousarray(
            xb.reshape(8, 128, LBLK).transpose(1, 0, 2)).astype(bf16)
        m["kgT"] = np.ascontiguousarray(kg_embeds[b].T.astype(f8))
        m["bge"] = col8(bge)
        in_maps.append(m)

    _CACHE["in_maps"] = in_maps
    res = run_bass_kernel_spmd(nc, in_maps, core_ids=list(range(N_CORES)))
    out = np.empty((B, L, D), np.float32)
    for c in range(N_CORES):
        b, j = divmod(c, 4)
        out[b, j * LBLK:(j + 1) * LBLK, :] = res.results[c]["OUTT"].T
    return out


# revision 44
# speedup vs baseline: 1.0122x; 1.0122x over previous
"""KGAttentionLayer Trainium2 kernel (v7).

Cost-model engine busy per core: ACT ~110us / DVE 115us / PE 112us
(PE ~82us real: the sim does not credit row-packed score concurrency) /
Pool ~68us -- down from v5's DVE 144 / ACT 140 / PE 128. The measured
4-core AllGather is ~19us for the 1.06MB payload and overlaps compute.

v7 adds position-sharded k/v projections with a 4-core AllGather:
- Sharding: 8 cores = (batch 2) x (query-block 4). Core c handles batch
  b=c//4, query rows [j*512, (j+1)*512) of that batch (j=c%4). The host
  rolls x per core so its queries AND its k/v "home" positions are
  columns [0, 512).
- Each core computes k/v projections only for its 512 home positions
  (all 16 heads), stages them to DRAM, and a rank-ordered AllGather over
  the batch group [[0..3],[4..7]] reassembles the full 2048-position k/v
  on every core (softmax is permutation-invariant over attended
  positions, and rank order == absolute order, so no per-core control
  flow is needed). kg-derived k/v (256 positions) are computed locally
  on every core (cheap). This removes the 4x duplication of k/v work:
  PE busy drops ~40us, and the ACT/DVE bias/copy stages shrink 4x.
- Elementwise work is split across ACT/DVE/Pool. The GPSIMD/Pool engine
  cannot touch PSUM (BIR verifier), so it takes the SBUF-only work:
  final gate/residual chain, recr copies, vslab ones. The 18.9M-element
  exp stream runs on ACT (true exp) and DVE (Schraudolph: the e4m3 bit
  pattern of exp(x) is int8(x*8/ln2 + 55.04), +-3% which cancels in the
  softmax ratio), split 9/9 per 18 m-chunks.
- fp8e4m3 DoubleRow matmuls for all projections and attn@v (weights
  pre-scaled 16x on host to clear e4m3 subnormals; 1/16 folded into the
  bias stage; for v the 16 is cancelled by a 16.0 ones-column in vslab
  that also emits the softmax denominator row).
- v-bias is NOT applied in vslab: out = num/den + bv, so bv folds into
  the out-proj/gate biases on the host (Wo@bv, Wg1@bv); kg rows carry
  (bkv - bv) baked in on-device.
- Row-packed K=64 score matmuls: head pairs stacked on partitions
  0:64/64:128 run concurrently in separate PE row groups.

Layouts (host pre-transposes; the PE never transposes on device):
  xT      [1024, 512]   x[b].T home slice (rolled so the core's block is
                        at columns [0, 512))
  qts2    [128, 8, 512]  pair-stacked q (even head dims at parts 0:64)
  ktaG    [128, 8, 2304] pair-stacked k, all pairs x all positions
                         (cols 0:2048 gathered, 2048:2304 local kg)
  vslabG  [128, 18, 16*68] per-head 64 v-cols plus a 16.0 ones column
          -> the attn@v matmul emits the softmax numerator in PSUM rows
          0..63 and the denominator in row 64.
"""

import sys

sys.path.insert(0, "/opt/trn_rl_repo")

import numpy as np

import concourse.bass as bass
import concourse.mybir as mybir
import concourse.tile as tile
from concourse import bacc
from concourse.bass_utils import run_bass_kernel_spmd

F32 = mybir.dt.float32
BF16 = mybir.dt.bfloat16
FP8 = mybir.dt.float8e4
DR = mybir.MatmulPerfMode.DoubleRow
AF = mybir.ActivationFunctionType
OP = mybir.AluOpType

WSCALE = 16.0
RSCALE = 1.0 / WSCALE

D = 1024
H = 16
HD = 64
B = 2
L = 2048
E = 256
LBLK = 512          # queries (and home k/v positions) per core
M = L + E           # 2304 attended positions
NMC = M // 128      # 18 m-chunks
HMC = LBLK // 128   # 4 home m-chunks
VW = 68             # vslab per-head slot (65 used; padded for alignment)
N_CORES = 8
GROUPS = [[0, 1, 2, 3], [4, 5, 6, 7]]

# Per-mc exp engine assignment: 'A' = ACT true exp, 'D' = DVE Schraudolph.
# Pairs 6-7 shift one tile to DVE (70 A / 74 D overall) to balance ACT's
# bias-stage load against DVE's reciprocal/copy load.
EXP_PAT = ['D', 'A', 'D', 'A', 'D', 'A', 'D', 'A', 'D',
           'A', 'D', 'A', 'D', 'A', 'D', 'A', 'D', 'A']
EXP_PAT2 = ['D', 'A', 'D', 'A', 'D', 'A', 'D', 'A', 'D',
            'A', 'D', 'A', 'D', 'A', 'D', 'D', 'D', 'A']
assert len(EXP_PAT) == len(EXP_PAT2) == NMC

_CACHE = {}


def _build(repeat=1):
    nc = bacc.Bacc("TRN2", target_bir_lowering=False, debug=False,
                   num_devices=N_CORES)

    dram = {}

    def din(name, shape, dt=BF16):
        dram[name] = nc.dram_tensor(name, shape, dt, kind="ExternalInput")
        return dram[name]

    xT = din("xT", [D, LBLK], FP8)
    xR = din("xR", [128, 8, LBLK], BF16)   # residual slice of x
    kgT = din("kgT", [D, E], FP8)
    WqT = din("WqT", [D, D], FP8)
    WkT = din("WkT", [D, D], FP8)
    WkkT = din("WkkT", [D, D], FP8)
    WvT = din("WvT", [D, D], FP8)
    WkvT = din("WkvT", [D, D], FP8)
    WoT = din("WoT", [D, D], FP8)
    WgT = din("WgT", [D, D], FP8)
    bq = din("bq", [128, 8], F32)      # col g = (bias*0.125)[g*128:(g+1)*128]
    bk = din("bk", [128, 8], F32)
    bkk = din("bkk", [128, 8], F32)
    bo = din("bo", [128, 8], F32)      # bo + Wo @ bv (v-bias folded on host)
    bge = din("bge", [128, 8], F32)    # bg + Wg[:,D:] @ kg_mean + Wg[:,:D] @ bv
    bkvb = din("bkvb", [128, D], F32)  # np.tile((bkv - bv) * 16, (128, 1))

    OUTT = nc.dram_tensor("OUTT", [D, LBLK], F32, kind="ExternalOutput")

    # DRAM scratch for the k/v exchange: one fp8 payload carrying both
    # the home k (8 pairs x 512 positions, as 16*(k+bk)) and the home
    # vslab (4 m-chunks x 16*VW).
    KW = 8 * LBLK                  # 4096 kta bytes/partition
    SW = KW + HMC * H * VW         # + 4*1088 vslab bytes -> 8448
    kv_out = nc.dram_tensor("kv_out", [128, SW], FP8)
    kv_g = nc.dram_tensor("kv_g", [4, 128, SW], FP8)
    # bounce buffer for the per-pair softmax reciprocals: a stride-0 DMA
    # re-reads the row 64x to broadcast it across partitions, so the
    # normalize muls can run on the (PSUM-blocked but idle) Pool engine
    recd = nc.dram_tensor("recd", [8, 1024], F32)

    def w8(pool, W, g, tag="w8"):
        t = pool.tile([128, 8, 128], FP8, tag=tag, name=f"{tag}_{W.name}_{g}")
        nc.sync.dma_start(
            t[:], W.ap().rearrange("(kk p) d -> p kk d", p=128)
            [:, :, g * 128:(g + 1) * 128])
        return t

    def wv4(pool, W, di, tag="wv"):
        t = pool.tile([128, 8, 256], FP8, tag=tag, name=f"{tag}_{W.name}_{di}")
        nc.gpsimd.dma_start(
            t[:], W.ap().rearrange("(kk p) d -> p kk d", p=128)
            [:, :, di * 256:(di + 1) * 256])
        return t

    from contextlib import ExitStack

    with tile.TileContext(nc) as tc, ExitStack() as ctx:
        persist = ctx.enter_context(tc.tile_pool(name="persist", bufs=1))
        kvpool = ctx.enter_context(tc.tile_pool(name="kvpool", bufs=2))
        wpool = ctx.enter_context(tc.tile_pool(name="wpool", bufs=2))
        spool = ctx.enter_context(tc.tile_pool(name="spool", bufs=2))
        epool = ctx.enter_context(tc.tile_pool(name="epool", bufs=3))
        # PSUM: scores 2x[128,1024] (4 banks) + av 1x[65,1024] (2 banks)
        #       + proj 2x[128,512] (2 banks) = 8 banks
        psS = ctx.enter_context(tc.tile_pool(name="psS", bufs=2, space="PSUM"))
        psAV = ctx.enter_context(tc.tile_pool(name="psAV", bufs=1, space="PSUM"))
        psP = ctx.enter_context(tc.tile_pool(name="psP", bufs=2, space="PSUM"))

        # ---- resident loads ----
        xts = persist.tile([128, 8, LBLK], FP8, tag="xts")
        nc.sync.dma_start(xts[:], xT.ap().rearrange("(kk p) l -> p kk l", p=128))
        xrs = persist.tile([128, 8, LBLK], BF16, tag="xrs")
        nc.sync.dma_start(xrs[:], xR.ap())
        kgts = persist.tile([128, 8, E], FP8, tag="kgts")
        nc.sync.dma_start(kgts[:], kgT.ap().rearrange("(kk p) e -> p kk e", p=128))
        biases = {}
        for nm in ("bq", "bk", "bkk", "bo", "bge"):
            t = persist.tile([128, 8], F32, tag=nm, name=nm + "_sb")
            nc.sync.dma_start(t[:], dram[nm].ap())
            biases[nm] = t
        bkvbs = persist.tile([128, D], F32, tag="bkvbs")
        nc.sync.dma_start(bkvbs[:], bkvb.ap())
        onesvH = persist.tile([128, HMC, H, 1], FP8, tag="onesvH")
        nc.vector.memset(onesvH[:], WSCALE)
        onesvE = persist.tile([128, 2, H, 1], FP8, tag="onesvE")
        nc.vector.memset(onesvE[:], WSCALE)


        qts2 = persist.tile([128, 8, LBLK], BF16, tag="qts2")
        outTs = persist.tile([128, 8, LBLK], FP8, tag="outTs")

        # repeat>1 builds a NEFF that runs the whole body `repeat` times --
        # used only by the timing harness (slope removes the RPC overhead).
        for _rep in range(repeat):
            # ---- home k projection: 8 pairs x home 512 positions ----
            # k is exchanged and kept in fp8 as 16*(k+bk); the 1/16 is
            # folded into the q scale, keeping e4m3 values normal.
            ktaH = kvpool.tile([128, 8, LBLK], FP8, tag="ktaH")
            for g in range(8):
                wk = w8(wpool, WkT, g, tag="wk")
                ps = psP.tile([128, LBLK], F32, tag="psP", name=f"khp{g}")
                for t in range(4):
                    nc.tensor.matmul(ps[:], wk[:, 2 * t:2 * t + 2, :],
                                     xts[:, 2 * t:2 * t + 2, :],
                                     start=(t == 0), stop=(t == 3),
                                     perf_mode=DR)
                nc.scalar.activation(ktaH[:, g, :], ps[:], AF.Identity,
                                     bias=biases["bk"][:, g:g + 1])

            # ---- home v projection: 16 heads x home positions ----
            vslabH = kvpool.tile([128, HMC, H * VW], FP8, tag="vslabH")
            nc.gpsimd.tensor_copy(
                vslabH[:].rearrange("p mc (h c) -> p mc h c", c=VW)
                [:, :, :, 64:65], onesvH[:])
            for di in range(4):
                wv = wv4(wpool, WvT, di)
                for mc in range(HMC):
                    ps = psP.tile([128, 256], F32, tag="psP",
                                  name=f"vhp{di}_{mc}")
                    for t in range(4):
                        nc.tensor.matmul(
                            ps[:], xts[:, 2 * t:2 * t + 2,
                                       mc * 128:(mc + 1) * 128],
                            wv[:, 2 * t:2 * t + 2, :],
                            start=(t == 0), stop=(t == 3), perf_mode=DR)
                    nc.vector.tensor_copy(
                        vslabH[:, mc, :].rearrange("p (h c) -> p h c", c=VW)
                        [:, 4 * di:4 * di + 4, 0:64],
                        ps[:].rearrange("p (h c) -> p h c", c=64))

            # ---- stage home k/v and gather across the batch group ----
            nc.sync.dma_start(
                kv_out.ap()[:, 0:KW].rearrange("p (g m) -> p g m", g=8),
                ktaH[:])
            nc.sync.dma_start(
                kv_out.ap()[:, KW:SW].rearrange("p (c w) -> p c w", c=HMC),
                vslabH[:])
            nc.gpsimd.collective_compute(
                "AllGather", OP.bypass, replica_groups=GROUPS,
                ins=[kv_out.ap()], outs=[kv_g.ap()])

            # ---- local kg projections (all cores, cheap) ----
            ktaG = kvpool.tile([128, 8, M], FP8, tag="ktaG")
            vslabG = kvpool.tile([128, NMC, H * VW], FP8, tag="vslabG")
            for g in range(8):
                wkk = w8(wpool, WkkT, g, tag="wk")
                ps = psP.tile([128, E], F32, tag="psP", name=f"kkp{g}")
                for t in range(4):
                    nc.tensor.matmul(ps[:], wkk[:, 2 * t:2 * t + 2, :],
                                     kgts[:, 2 * t:2 * t + 2, :],
                                     start=(t == 0), stop=(t == 3),
                                     perf_mode=DR)
                nc.scalar.activation(ktaG[:, g, L:M], ps[:], AF.Identity,
                                     bias=biases["bkk"][:, g:g + 1])
            nc.gpsimd.tensor_copy(
                vslabG[:, 16:18, :].rearrange("p mc (h c) -> p mc h c", c=VW)
                [:, :, :, 64:65], onesvE[:])
            for di in range(4):
                wkv = wv4(wpool, WkvT, di, tag="wkv")
                for emc in range(2):
                    ps = psP.tile([128, 256], F32, tag="psP",
                                  name=f"kvp{di}_{emc}")
                    for t in range(4):
                        nc.tensor.matmul(
                            ps[:], kgts[:, 2 * t:2 * t + 2,
                                        emc * 128:(emc + 1) * 128],
                            wkv[:, 2 * t:2 * t + 2, :],
                            start=(t == 0), stop=(t == 3), perf_mode=DR)
                    nc.vector.tensor_add(
                        vslabG[:, 16 + emc, :]
                        .rearrange("p (h c) -> p h c", c=VW)
                        [:, 4 * di:4 * di + 4, 0:64],
                        ps[:].rearrange("p (h c) -> p h c", c=64),
                        bkvbs[:, 256 * di:256 * di + 256]
                        .rearrange("p (h c) -> p h c", c=64))

            # ---- read back the gathered k/v (rank order == absolute) ----
            # gpsimd queue: keeps the readbacks off the sync queue (which
            # carries the weight streams) so attention can start as soon
            # as the gather lands.
            for r in range(4):
                nc.gpsimd.dma_start(
                    ktaG[:, :, r * LBLK:(r + 1) * LBLK],
                    kv_g.ap()[r][:, 0:KW].rearrange("p (g m) -> p g m", g=8))
                nc.gpsimd.dma_start(
                    vslabG[:, 4 * r:4 * r + 4, :],
                    kv_g.ap()[r][:, KW:SW].rearrange("p (c w) -> p c w",
                                                     c=HMC))

            # ---- q projections (all 8 pairs) ----
            for g in range(8):
                wq = w8(wpool, WqT, g, tag="wq")
                ps = psP.tile([128, LBLK], F32, tag="psP", name=f"qp{g}")
                for t in range(4):
                    nc.tensor.matmul(ps[:], wq[:, 2 * t:2 * t + 2, :],
                                     xts[:, 2 * t:2 * t + 2, :],
                                     start=(t == 0), stop=(t == 3),
                                     perf_mode=DR)
                # scale = attn 0.125 / 16 (Wq prescale) / 16 (kta carries
                # 16*(k+bk)); host scales bq to match
                nc.scalar.activation(qts2[:, g, :], ps[:], AF.Identity,
                                     bias=biases["bq"][:, g:g + 1],
                                     scale=0.125 * RSCALE * RSCALE)

            # ---- attention per pair g (heads 2g, 2g+1) ----
            for g in range(8):
                avp = psAV.tile([65, 1024], F32, tag="psAV", name=f"av{g}")
                for mc in range(NMC):
                    sp = psS.tile([128, 1024], F32, tag="psS",
                                  name=f"sp{g}_{mc}")
                    # two concurrent K=64 matmuls in different PE row groups
                    nc.tensor.matmul(sp[:, 0:512],
                                     ktaG[0:64, g, mc * 128:(mc + 1) * 128],
                                     qts2[0:64, g, :], start=True, stop=True)
                    nc.tensor.matmul(sp[:, 512:1024],
                                     ktaG[64:128, g, mc * 128:(mc + 1) * 128],
                                     qts2[64:128, g, :], start=True, stop=True)
                    if mc % 2 == 0:
                        et2 = epool.tile([128, 2, 1024], FP8, tag="et",
                                         name=f"et{g}_{mc}")
                    if (EXP_PAT if g < 6 else EXP_PAT2)[mc] == 'A':
                        nc.scalar.activation(et2[:, mc % 2, :], sp[:], AF.Exp)
                    else:
                        nc.vector.tensor_scalar(
                            et2[:, mc % 2, :].bitcast(mybir.dt.uint8),
                            sp[:], 11.5416, 55.04, OP.mult, OP.add)
                    if mc % 2 == 1:
                        # DoubleRow fp8 av over the two chunks of et2
                        nc.tensor.matmul(
                            avp[:, 0:512],
                            vslabG[:, mc - 1:mc + 1,
                                   2 * g * VW:2 * g * VW + 65],
                            et2[:, :, 0:512],
                            start=(mc == 1), stop=(mc == NMC - 1),
                            perf_mode=DR)
                        nc.tensor.matmul(
                            avp[:, 512:1024],
                            vslabG[:, mc - 1:mc + 1,
                                   (2 * g + 1) * VW:(2 * g + 1) * VW + 65],
                            et2[:, :, 512:1024],
                            start=(mc == 1), stop=(mc == NMC - 1),
                            perf_mode=DR)
                # post-process: reciprocal of denominators, broadcast via PE,
                # normalize on DVE into outTs.
                # (reciprocal_approx_fast reads garbage from PSUM on HW --
                # stage the denominator row in SBUF first.)
                # one ACT copy moves numerator rows AND the denominator row
                # (adjacent PSUM partitions 0:65) to SBUF in a single op.
                # reciprocal_approx_fast needs a base-partition-0 input
                # (it reads garbage at an offset base partition, like it
                # does from PSUM) -> bounce the den row via Pool.
                avd = spool.tile([65, 1024], F32, tag="avd", name=f"avd{g}")
                nc.scalar.activation(avd[:], avp[0:65, :], AF.Identity)
                den = spool.tile([1, 1024], F32, tag="den", name=f"den{g}")
                nc.gpsimd.tensor_copy(den[:], avd[64:65, :])
                rec = spool.tile([1, 1024], F32, tag="rec", name=f"rec{g}")
                nc.vector.reciprocal_approx_fast(rec[:], den[:])
                # broadcast rec across 64 partitions via a stride-0 DMA
                # (DRAM bounce), then normalize on Pool (all-SBUF)
                nc.sync.dma_start(recd.ap()[g], rec[:])
                recb = spool.tile([64, 1024], F32, tag="recb",
                                  name=f"recb{g}")
                rsrc = recd.ap()[g]
                nc.sync.dma_start(
                    recb[:],
                    bass.AP(tensor=rsrc.tensor, offset=rsrc.offset,
                            ap=[[0, 64], [1, 1024]]))
                nc.gpsimd.tensor_mul(outTs[0:64, g, :], avd[0:64, 0:512],
                                     recb[:, 0:512])
                nc.gpsimd.tensor_mul(outTs[64:128, g, :],
                                     avd[0:64, 512:1024],
                                     recb[:, 512:1024])

            # ---- out-proj + gate + residual ----
            for g in range(8):
                wo = w8(wpool, WoT, g, tag="wo")
                wg = w8(wpool, WgT, g, tag="wo")
                pp = psP.tile([128, LBLK], F32, tag="psP", name=f"pp{g}")
                for t in range(4):
                    nc.tensor.matmul(pp[:], wo[:, 2 * t:2 * t + 2, :],
                                     outTs[:, 2 * t:2 * t + 2, :],
                                     start=(t == 0), stop=(t == 3),
                                     perf_mode=DR)
                pj = spool.tile([128, LBLK], F32, tag="pj", name=f"pj{g}")
                nc.scalar.activation(pj[:], pp[:], AF.Identity,
                                     bias=biases["bo"][:, g:g + 1],
                                     scale=RSCALE)
                gp = psP.tile([128, LBLK], F32, tag="psP", name=f"gp{g}")
                for t in range(4):
                    nc.tensor.matmul(gp[:], wg[:, 2 * t:2 * t + 2, :],
                                     outTs[:, 2 * t:2 * t + 2, :],
                                     start=(t == 0), stop=(t == 3),
                                     perf_mode=DR)
                gt = spool.tile([128, LBLK], F32, tag="gt", name=f"gt{g}")
                nc.scalar.activation(gt[:], gp[:], AF.Sigmoid,
                                     bias=biases["bge"][:, g:g + 1],
                                     scale=RSCALE)
                # all-SBUF chain -> Pool engine (cannot touch PSUM)
                d1 = spool.tile([128, LBLK], F32, tag="fin", name=f"d1{g}")
                nc.gpsimd.tensor_sub(d1[:], pj[:], xrs[:, g, :])
                d2 = spool.tile([128, LBLK], F32, tag="fin", name=f"d2{g}")
                nc.gpsimd.tensor_mul(d2[:], d1[:], gt[:])
                fo = spool.tile([128, LBLK], F32, tag="fin", name=f"fo{g}")
                nc.gpsimd.tensor_add(fo[:], d2[:], xrs[:, g, :])
                nc.sync.dma_start(OUTT.ap()[g * 128:(g + 1) * 128, :], fo[:])

    nc.compile()
    return nc


def kernel(x, kg_embeds, Wq, bq, Wk, bk, Wv, bv, Wkk, bkk, Wkv, bkv,
           Wo, bo, Wg, bg):
    import ml_dtypes
    bf16 = ml_dtypes.bfloat16
    f8 = ml_dtypes.float8_e4m3

    x = np.asarray(x, np.float32)
    kg_embeds = np.asarray(kg_embeds, np.float32)
    ws = {k: np.asarray(v, np.float32) for k, v in dict(
        Wq=Wq, bq=bq, Wk=Wk, bk=bk, Wv=Wv, bv=bv, Wkk=Wkk, bkk=bkk,
        Wkv=Wkv, bkv=bkv, Wo=Wo, bo=bo, Wg=Wg, bg=bg).items()}

    if "nc" not in _CACHE:
        _CACHE["nc"] = _build()
    nc = _CACHE["nc"]

    def col8(v):
        return np.ascontiguousarray(v.reshape(8, 128).T)

    def w16(w):
        # weights pre-scaled by WSCALE=16 before fp8 so they stay normal
        return np.ascontiguousarray((w.T * 16.0).astype(f8))

    shared = {
        "WqT": w16(ws["Wq"]),
        "WkT": w16(ws["Wk"]),
        "WkkT": w16(ws["Wkk"]),
        "WvT": w16(ws["Wv"]),
        "WkvT": w16(ws["Wkv"]),
        "WoT": w16(ws["Wo"]),
        "WgT": w16(ws["Wg"][:, :D]),
        "bq": col8(ws["bq"] * (0.125 / WSCALE)),
        "bk": col8(ws["bk"] * WSCALE),
        "bkk": col8(ws["bkk"] * WSCALE),
        # vslab holds 16*(x@Wv.T) WITHOUT the v bias; bv is applied
        # exactly via the out-proj/gate biases:
        #   out = num/den + bv  =>  Wo@out = Wo@(num/den) + Wo@bv
        # kg rows carry the residual (bkv - bv) baked into vslab.
        "bo": col8(ws["bo"] + ws["Wo"] @ ws["bv"]),
        "bkvb": np.ascontiguousarray(
            np.tile((ws["bkv"] - ws["bv"]) * 16.0, (128, 1))),
    }

    in_maps = []
    for c in range(N_CORES):
        b, j = divmod(c, 4)
        # roll the core's query block (== its home k/v positions) to
        # columns [0, 512); only those columns of x are needed on-device
        xb = np.ascontiguousarray(np.roll(x[b].T, -j * LBLK, axis=1)[:, :LBLK])
        kgm = kg_embeds[b].mean(axis=0)
        bge = ws["bg"] + ws["Wg"][:, D:] @ kgm + ws["Wg"][:, :D] @ ws["bv"]
        m = dict(shared)
        m["xT"] = xb.astype(f8)
        m["xR"] = np.ascontiguousarray(
            xb.reshape(8, 128, LBLK).transpose(1, 0, 2)).astype(bf16)
        m["kgT"] = np.ascontiguousarray(kg_embeds[b].T.astype(f8))
        m["bge"] = col8(bge)
        in_maps.append(m)

    _CACHE["in_maps"] = in_maps
    res = run_bass_kernel_spmd(nc, in_maps, core_ids=list(range(N_CORES)))
    out = np.empty((B, L, D), np.float32)
    for c in range(N_CORES):
        b, j = divmod(c, 4)
        out[b, j * LBLK:(j + 1) * LBLK, :] = res.results[c]["OUTT"].T
    return out
